# revision 23
# baseline (speedup 1.0000x reference)
"""Trainium2 Bass kernel for nn_DE (retrieval_knn).

Problem: B=8 batches of L=4096 1-D points. For each query point, find the
index of its 2nd-closest pool point under stable (lowest-index) tie-break:
  - context rows [0, 2048): pool = rows [0, 2048)
  - target  rows [2048+i]:  pool = rows [0, 2048+i]  (causal band)
Then (host side, O(B*L) only): gather neighbors, finite-difference deriv,
clip, batch-norm (training stats), concat clip-label.

Device strategy (one NeuronCore per batch element, SPMD over 8 cores):
for each 128-query tile, ACT computes z = (x_pool - x_q)^2 against the
broadcast pool row, GPSIMD negates (+ applies the causal triangular mask on
the boundary block), and DVE max/max_index produce the stable top-2
smallest distances' indices in two passes. Index [1] is the answer.
Selection on z instead of d=sqrt(z) is exact here: sqrt is monotone and the
stable-tie selections agree on these inputs (verified bitwise vs the jax
reference).
"""

import sys

for _p in ("/opt/trn_rl_repo", "/root/.axon_site/_ro/trn_rl_repo"):
    if _p not in sys.path:
        sys.path.append(_p)

import numpy as np

B = 8
L = 4096
N_C = 2048
N_T = 2048
NTILE = L // 128  # 32 query tiles: 16 context + 16 target
BIG = 1.0e9

EPS_FD = np.float32(2e-6)
BN_EPS = np.float32(1e-3)

_CACHE = {}

# tuned on the TimelineSim cost model (192.5 us/core predicted;
# DVE busy floor for this ISA is 176.7 us)
NC_CONFIG = dict(zbufs=3, wbufs=3, pool_frac=0.75, bc_mode="hw8")


def _build_nc(zbufs=3, wbufs=3, pool_frac=0.75, bc_mode="pe"):
    import concourse.bass as bass
    import concourse.bacc as bacc
    import concourse.mybir as mybir
    from concourse.tile import TileContext

    F32 = mybir.dt.float32
    U32 = mybir.dt.uint32

    nc = bacc.Bacc(None, target_bir_lowering=False)
    xin = nc.declare_dram_parameter("xin", [L], F32, isOutput=False)
    ixo = nc.declare_dram_parameter("ix_out", [128, NTILE], U32, isOutput=True)

    # strict upper-triangular causal penalty for the boundary 128-block
    tri_np = np.triu(np.full((128, 128), BIG, np.float32), k=1)
    tri_d = nc.inline_tensor(tri_np, name="tri_const")

    with TileContext(nc) as tc:
        with (
            tc.tile_pool(name="consts", bufs=1) as cpool,
            tc.tile_pool(name="zp", bufs=zbufs) as zpool,
            tc.tile_pool(name="wp", bufs=wbufs) as wpool,
            tc.tile_pool(name="small", bufs=4) as spool,
            tc.tile_pool(name="psum", bufs=1, space="PSUM") as ppool,
        ):
            # triangular causal mask (BIG above diagonal), subtracted from w
            tri = cpool.tile([128, 128], F32, name="tri")
            nc.sync.dma_start(out=tri[:, :], in_=tri_d[:, :])

            # queries along partitions: xq[q, t] = x[128*t + q]
            xq = cpool.tile([128, NTILE], F32, name="xq")
            nc.sync.dma_start(
                out=xq[:, :], in_=xin[:].rearrange("(n p) -> p n", p=128)
            )

            # pool row broadcast across all 128 partitions
            if bc_mode == "pe_psum":
                # PE outer product IS the broadcast; squares read PSUM direct
                xrow = cpool.tile([1, L], F32, name="xrow")
                nc.gpsimd.dma_start(out=xrow[:, :], in_=xin[:].unsqueeze(0))
                ones_r = cpool.tile([1, 128], F32, name="ones_r")
                nc.gpsimd.memset(ones_r[:, :], 1.0)
                pool_bc = ppool.tile([128, L], F32, name="pool_ps")
                for c in range(0, L, 512):
                    nc.tensor.matmul(
                        pool_bc[:, c : c + 512], ones_r[:, :], xrow[:, c : c + 512]
                    )
            else:
                pool_bc = cpool.tile([128, L], F32, name="pool_bc")
            if bc_mode == "pe_psum":
                pass
            elif bc_mode == "pe":
                # PE rank-1 outer product ones[128] x row[L] -> PSUM -> SBUF
                xrow = cpool.tile([1, L], F32, name="xrow")
                nc.gpsimd.dma_start(out=xrow[:, :], in_=xin[:].unsqueeze(0))
                ones_r = cpool.tile([1, 128], F32, name="ones_r")
                nc.gpsimd.memset(ones_r[:, :], 1.0)
                pool_ps = ppool.tile([128, L], F32, name="pool_ps")
                for c in range(0, L, 512):
                    nc.tensor.matmul(
                        pool_ps[:, c : c + 512], ones_r[:, :], xrow[:, c : c + 512]
                    )
                nc.scalar.activation(
                    pool_bc[:, :],
                    pool_ps[:, :],
                    mybir.ActivationFunctionType.Copy,
                    bias=0.0,
                    scale=1.0,
                )
            elif bc_mode == "dma":
                nc.gpsimd.dma_start(
                    out=pool_bc[:, :],
                    in_=xin[:].unsqueeze(0).partition_broadcast(128),
                )
            elif bc_mode == "dma8":  # chunked broadcast over the SWDGE queues
                for c in range(0, L, 512):
                    nc.gpsimd.dma_start(
                        out=pool_bc[:, c : c + 512],
                        in_=xin[c : c + 512].unsqueeze(0).partition_broadcast(128),
                    )
            else:  # "hwN": chunked broadcast over the parallel HWDGE queues
                csz = int(bc_mode[2:]) if len(bc_mode) > 2 else 512
                csz = L // csz if csz else 512
                for c in range(0, L, csz):
                    nc.sync.dma_start(
                        out=pool_bc[:, c : c + csz],
                        in_=xin[c : c + csz].unsqueeze(0).partition_broadcast(128),
                    )

            negones = cpool.tile([128, L], F32, name="negones")
            nc.gpsimd.memset(negones[:, :], -1.0)

            # dep-free dummy activation: pulls LoadActFuncSet off the
            # first-tile critical path
            warm = cpool.tile([128, 8], F32, name="warm")
            nc.vector.memset(warm[:, :], 0.0)
            nc.scalar.activation(
                warm[:, :], warm[:, :], mybir.ActivationFunctionType.Square,
                bias=warm[:, 0:1], scale=-1.0,
            )

            # per-tile top-8 indices; column 1 of each 8-block is the answer
            out8 = cpool.tile([128, NTILE * 8], U32, name="out8")

            for t in range(NTILE):
                if t < 16:
                    P = N_C
                else:
                    P = N_C + 128 * (t - 16 + 1)
                # z = (xq - pool)^2 == Square(pool * (-1) + xq)
                z = zpool.tile([128, L], F32, name="z", tag="z")
                w = wpool.tile([128, L], F32, name="w", tag="w")
                if t < 2:
                    # chunked square (ACT) + negate (idle DVE), overlapping
                    # the arrival of the broadcast DMA chunks
                    for c in range(0, P, 512):
                        nc.scalar.activation(
                            z[:, c : c + 512],
                            pool_bc[:, c : c + 512],
                            mybir.ActivationFunctionType.Square,
                            bias=xq[:, t : t + 1],
                            scale=-1.0,
                        )
                        nc.vector.tensor_scalar_mul(
                            w[:, c : c + 512], z[:, c : c + 512], -1.0
                        )
                else:
                    nc.scalar.activation(
                        z[:, :P],
                        pool_bc[:, :P],
                        mybir.ActivationFunctionType.Square,
                        bias=xq[:, t : t + 1],
                        scale=-1.0,
                    )
                    # w = -z; split columns between Pool (TT) and ACT (Copy)
                    Pp = (int(P * pool_frac) // 128) * 128
                    if Pp > 0:
                        nc.gpsimd.tensor_tensor(
                            w[:, :Pp],
                            z[:, :Pp],
                            negones[:, :Pp],
                            op=mybir.AluOpType.mult,
                        )
                    if Pp < P:
                        nc.scalar.activation(
                            w[:, Pp:P],
                            z[:, Pp:P],
                            mybir.ActivationFunctionType.Copy,
                            bias=0.0,
                            scale=-1.0,
                        )
                if t >= 16:
                    # boundary block: w -= tri  (mask future positions)
                    nc.gpsimd.tensor_tensor(
                        w[:, P - 128 : P],
                        w[:, P - 128 : P],
                        tri[:, :],
                        op=mybir.AluOpType.subtract,
                    )
                mx = spool.tile([128, 8], F32, name="mx", tag="mx")
                nc.vector.max(mx[:, :], w[:, :P])
                nc.vector.max_index(out8[:, 8 * t : 8 * t + 8], mx[:, :], w[:, :P])

            nc.sync.dma_start(
                out=ixo[:, :],
                in_=out8[:, :].rearrange("p (t e) -> p t e", e=8)[:, :, 1],
            )
    nc.compile()  # bacc lowering: splits >1-wait instructions via event sems
    return nc


def get_nc(**kw):
    key = tuple(sorted(kw.items()))
    if key not in _CACHE:
        _CACHE[key] = _build_nc(**kw)
    return _CACHE[key]


def _device_indices(x_flat: np.ndarray) -> np.ndarray:
    """Run the bass kernel on 8 cores. x_flat: [B, L] f32 -> ix [B, L] int64."""
    from concourse.bass_utils import run_bass_kernel_spmd

    nc = get_nc(**NC_CONFIG)
    in_maps = [{"xin": np.ascontiguousarray(x_flat[b])} for b in range(B)]
    res = run_bass_kernel_spmd(nc, in_maps, list(range(B))).results
    ix = np.zeros((B, L), np.int64)
    for b in range(B):
        out = np.asarray(res[b]["ix_out"])  # [128, NTILE] uint32
        ix[b] = out.T.reshape(-1)  # flat[128*t + q] = out[q, t]
    return ix


def kernel(y, x, gamma, beta, n_C, n_T, training):
    y = np.asarray(y, dtype=np.float32)
    x = np.asarray(x, dtype=np.float32)
    gamma = np.asarray(gamma, dtype=np.float32)
    beta = np.asarray(beta, dtype=np.float32)
    assert x.shape == (B, L, 1) and int(n_C) == N_C and int(n_T) == N_T

    ix = _device_indices(x[:, :, 0])

    # ---- host tail: O(B*L) elementwise + batchnorm ----
    x_cl = np.take_along_axis(x, ix[..., None], axis=1)
    y_cl = np.take_along_axis(y, ix[..., None], axis=1)
    x_rep = (x - x_cl).astype(np.float32)
    y_rep = (y - y_cl).astype(np.float32)
    dist = np.sqrt(
        (x_rep * x_rep).astype(np.float32).sum(-1, keepdims=True).astype(np.float32)
    ).astype(np.float32)
    deriv = (y_rep / (EPS_FD + dist)).astype(np.float32)
    deriv_new = np.where(np.isnan(deriv), np.float32(10000.0), deriv)
    deriv_new2 = np.where(
        np.abs(deriv_new) > np.float32(200.0), np.float32(0.0), deriv_new
    )
    if int(training):
        mean = deriv_new2.mean(axis=(0, 1), keepdims=True, dtype=np.float32)
        var = ((deriv_new2 - mean) ** 2).mean(axis=(0, 1), keepdims=True, dtype=np.float32)
    else:
        mean = np.zeros((1, 1, deriv_new2.shape[-1]), np.float32)
        var = np.ones((1, 1, deriv_new2.shape[-1]), np.float32)
    inv = (np.float32(1.0) / np.sqrt(var + BN_EPS)).astype(np.float32)
    deriv_scaled = ((deriv_new2 - mean) * inv * gamma + beta).astype(np.float32)
    label = (deriv_new2 == deriv_new).astype(np.float32)
    deriv_out = np.concatenate([deriv_scaled, label], axis=-1)
    return y_rep, x_rep, deriv_out, x_cl, y_cl
